# revision 11
# baseline (speedup 1.0000x reference)
"""Distributed Sinkhorn contrastive loss (Classify-Anything) on 8 Trainium2 cores.

Math: reference computes K = exp((exp(z/tau) - 1)/tau) (masked diagonal),
then 25 alternating row/col normalizations of K, then
loss = -mean(log p[i, sigma(i)]) over the two +-2048 diagonals.

Key factorization: alternating normalizations keep p = diag(u) K diag(v), so
only matrix-VECTOR products against the resident K are needed:
  u <- 1/(K v)   [rownorm]      v <- 1/(K^T u)  [colnorm]
(13 rownorms, 12 colnorms). Final loss needs only log u, log v, and the 4096
diagonal entries log K[i, sigma(i)] = (exp(10 z_ii') - 1)*10 from direct dots.

Sharding: rows are sharded 512/core. Each core holds its row-block of K twice:
A = [128i x 4096j] bf16 tiles (x4) and B = A^T as [128j x 512i] bf16 tiles
(x32, built by PE transpose). u-updates are fully local (full rows resident);
v-updates produce partial column sums -> one 16KB AllReduce per colnorm
(12 total). The z-GEMM runs in fp32 on the tensor engine; exponentials on the
scalar engine; diagonal masking via iota + is_equal + copy_predicated with a
per-core row-index input (no core-dependent addressing in the SPMD program).

fp32 overflow semantics: exp((s-1)*10) overflows fp32 for s > 9.87; the
reference then yields nan (inf -> rownorm -> nan -> spreads). The device path
reproduces this naturally (ACT exp emits real inf), and the host epilogue also
emulates fp32 overflow from the kernel-reported max(s) so the masked-diagonal
case matches too.
"""
import sys

sys.path.insert(0, "/opt/trn_rl_repo")

import numpy as np

N = 4096
D = 300
NCORES = 8
R = N // NCORES          # 512 rows per core
BATCH = N // 2           # 2048
NT = R // 128            # 4 row tiles per core
NS = N // 128            # 32 column tiles
ITER_PAIRS = 12          # SINKHORN_ITERS=25 -> 1 + 12*(colnorm, rownorm)

_CACHED = {}


def _build():
    import concourse.bacc as bacc
    import concourse.mybir as mybir
    import concourse.tile as tile

    F32 = mybir.dt.float32
    F32R = mybir.dt.float32r
    BF16 = mybir.dt.bfloat16
    U8 = mybir.dt.uint8
    AF = mybir.ActivationFunctionType
    ALU = mybir.AluOpType

    nc = bacc.Bacc("TRN2", target_bir_lowering=False, debug=False,
                   num_devices=NCORES)

    ftT_in = nc.dram_tensor("ftT", [D, R], F32R, kind="ExternalInput")
    ltT_in = nc.dram_tensor("ltT", [D, N], F32R, kind="ExternalInput")
    fblk_in = nc.dram_tensor("fblk", [R, D], F32, kind="ExternalInput")
    lsig_in = nc.dram_tensor("lsig", [R, D], F32, kind="ExternalInput")
    rga_in = nc.dram_tensor("rga", [128, NT], F32, kind="ExternalInput")
    ident_in = nc.dram_tensor("ident", [128, 128], F32, kind="ExternalInput")
    out_dram = nc.dram_tensor("out", [128, 4], F32, kind="ExternalOutput")

    DCH = [(0, 128), (128, 128), (256, D - 256)]  # contraction chunks of 300

    with tile.TileContext(nc) as tc:
        with (
            tc.tile_pool(name="sb", bufs=1) as sb,
            tc.tile_pool(name="ps", bufs=1, space="PSUM") as ps,
            tc.tile_pool(name="dram", bufs=1, space="DRAM") as dram,
        ):
            # ---------------- static inputs -> SBUF ----------------
            ft = [sb.tile([dn, R], F32R, name=f"ft{i}") for i, (d0, dn) in enumerate(DCH)]
            lt = [sb.tile([dn, N], F32R, name=f"lt{i}") for i, (d0, dn) in enumerate(DCH)]
            for i, (d0, dn) in enumerate(DCH):
                nc.sync.dma_start(ft[i][:], ftT_in[d0:d0 + dn, :])
                nc.sync.dma_start(lt[i][:], ltT_in[d0:d0 + dn, :])
            fb = [sb.tile([128, D], F32, name=f"fb{t}") for t in range(NT)]
            ls = [sb.tile([128, D], F32, name=f"ls{t}") for t in range(NT)]
            for t in range(NT):
                nc.sync.dma_start(fb[t][:], fblk_in[t * 128:(t + 1) * 128, :])
                nc.sync.dma_start(ls[t][:], lsig_in[t * 128:(t + 1) * 128, :])
            rga = sb.tile([128, NT], F32)
            nc.sync.dma_start(rga[:], rga_in[:])
            ident = sb.tile([128, 128], F32)
            nc.sync.dma_start(ident[:], ident_in[:])
            id16 = sb.tile([128, 128], BF16)
            nc.vector.tensor_copy(id16[:], ident[:])

            colidx = sb.tile([128, N], F32)
            nc.gpsimd.iota(colidx[:], pattern=[[1, N]], base=0,
                           channel_multiplier=0,
                           allow_small_or_imprecise_dtypes=True)
            zero16 = sb.tile([128, 1], BF16)
            nc.vector.memset(zero16[:], 0.0)
            bm10 = sb.tile([128, 1], F32)
            nc.vector.memset(bm10[:], -10.0)
            onesf = sb.tile([128, 1], F32)
            nc.vector.memset(onesf[:], 1.0)

            # ---------------- persistent state tiles ----------------
            A = [sb.tile([128, N], BF16, name=f"A{t}") for t in range(NT)]
            Bt = [sb.tile([128, R], BF16, name=f"B{s}") for s in range(NS)]
            mx = sb.tile([128, NS], F32)
            v32 = sb.tile([128, NS], BF16)
            nc.vector.memset(v32[:], 1.0)
            u16 = sb.tile([128, NT], BF16)
            lnu = sb.tile([128, NT], F32)
            lnv = sb.tile([128, NS], F32)
            usum = sb.tile([1, R], BF16)
            vpart = sb.tile([1, N], F32)
            v32raw = sb.tile([NS, 128], F32)
            v32r16 = sb.tile([NS, 128], BF16)

            # ---------------- phase 1: z-GEMM + double exp + mask ----------
            for ic in range(NT):
                for jc in range(8):
                    psz = ps.tile([128, 512], F32, tag="ps", bufs=6, name="psz")
                    for d in range(3):
                        nc.tensor.matmul(
                            psz[:],
                            ft[d][:, ic * 128:(ic + 1) * 128],
                            lt[d][:, jc * 512:(jc + 1) * 512],
                            start=(d == 0), stop=(d == 2),
                        )
                    stile = sb.tile([128, 512], F32, tag="stile", bufs=3,
                                    name="stile")
                    nc.scalar.activation(stile[:], psz[:], AF.Exp, scale=10.0)
                    nc.vector.tensor_reduce(
                        mx[:, ic * 8 + jc:ic * 8 + jc + 1], stile[:],
                        axis=mybir.AxisListType.X, op=ALU.max)
                    nc.scalar.activation(
                        A[ic][:, jc * 512:(jc + 1) * 512], stile[:], AF.Exp,
                        bias=bm10[:, 0:1], scale=10.0)
                # mask the diagonal stripe of this row tile (col == rga[:, ic])
                dm = sb.tile([128, N], U8, tag="dm", bufs=2, name="dm")
                nc.vector.tensor_scalar(
                    out=dm[:], in0=colidx[:], scalar1=rga[:, ic:ic + 1],
                    scalar2=None, op0=ALU.is_equal)
                nc.vector.copy_predicated(
                    A[ic][:], dm[:], zero16[:, 0:1].broadcast_to((128, N)))

            # ---------------- phase 2: B = A^T via PE transposes ----------
            for s in range(NS):
                for ic in range(NT):
                    pst = ps.tile([128, 128], BF16, tag="ps", bufs=6, name="pst")
                    nc.tensor.transpose(
                        pst[:], A[ic][:, s * 128:(s + 1) * 128], id16[:])
                    eng = nc.vector if (s * NT + ic) % 2 == 0 else nc.scalar
                    if eng is nc.vector:
                        eng.tensor_copy(Bt[s][:, ic * 128:(ic + 1) * 128], pst[:])
                    else:
                        eng.copy(Bt[s][:, ic * 128:(ic + 1) * 128], pst[:])

            # ---------------- diagonal term: zd = F_i . L_sigma(i) --------
            zdiag = sb.tile([128, NT], F32)
            for t in range(NT):
                tmp = sb.tile([128, D], F32, tag="tmp", bufs=2, name="tmp")
                nc.vector.tensor_mul(tmp[:], fb[t][:], ls[t][:])
                nc.vector.tensor_reduce(zdiag[:, t:t + 1], tmp[:],
                                        axis=mybir.AxisListType.X, op=ALU.add)
            sdiag = sb.tile([128, NT], F32)
            nc.scalar.activation(sdiag[:], zdiag[:], AF.Exp, scale=10.0)
            t2 = sb.tile([128, NT], F32)
            nc.vector.tensor_scalar(
                out=t2[:], in0=sdiag[:], scalar1=10.0, scalar2=10.0,
                op0=ALU.mult, op1=ALU.subtract)

            # ---------------- phase 3: Sinkhorn iterations ----------------
            def u_update(last: bool):
                # rowsums of my rows: accumulate over all 32 column chunks
                psu = ps.tile([1, R], F32, tag="ps", bufs=6, name="psu")
                for s in range(NS):
                    nc.tensor.matmul(psu[:], v32[:, s:s + 1], Bt[s][:],
                                     start=(s == 0), stop=(s == NS - 1))
                nc.scalar.copy(usum[:], psu[:])
                for t in range(NT):
                    put = ps.tile([128, 1], BF16, tag="ps", bufs=6, name="put")
                    nc.tensor.transpose(
                        put[:], usum[0:1, t * 128:(t + 1) * 128], id16[0:1, 0:1])
                    with nc.allow_low_precision(reason="bf16 sinkhorn scaling"):
                        nc.vector.reciprocal(u16[:, t:t + 1], put[:])
                    if last:
                        nc.scalar.activation(lnu[:, t:t + 1], put[:], AF.Ln)

            def v_update(last: bool):
                # partial colsums over my 512 rows, then AllReduce
                arin = dram.tile([1, N], F32, tag="arin", bufs=2, name="arin")
                arout = dram.tile([1, N], F32, addr_space="Shared",
                                  tag="arout", bufs=2, name="arout")
                for jc in range(8):
                    psv = ps.tile([1, 512], F32, tag="ps", bufs=6, name="psv")
                    for ic in range(NT):
                        nc.tensor.matmul(
                            psv[:], u16[:, ic:ic + 1],
                            A[ic][:, jc * 512:(jc + 1) * 512],
                            start=(ic == 0), stop=(ic == NT - 1))
                    if jc % 2 == 0:
                        nc.vector.tensor_copy(
                            vpart[:, jc * 512:(jc + 1) * 512], psv[:])
                    else:
                        nc.scalar.copy(
                            vpart[:, jc * 512:(jc + 1) * 512], psv[:])
                nc.sync.dma_start(arin[:], vpart[:])
                nc.gpsimd.collective_compute(
                    "AllReduce", ALU.add,
                    replica_groups=[list(range(NCORES))],
                    ins=[arin[:].opt()], outs=[arout[:].opt()])
                nc.sync.dma_start(
                    v32raw[:], arout[:].rearrange("a (b c) -> (a b) c", c=128))
                nc.vector.tensor_copy(v32r16[:], v32raw[:])
                pvt = ps.tile([128, NS], BF16, tag="ps", bufs=6, name="pvt")
                nc.tensor.transpose(pvt[:], v32r16[:], id16[0:NS, 0:NS])
                with nc.allow_low_precision(reason="bf16 sinkhorn scaling"):
                    nc.vector.reciprocal(v32[:], pvt[:])
                if last:
                    nc.scalar.activation(lnv[:], pvt[:], AF.Ln)

            u_update(last=(ITER_PAIRS == 0))
            for it in range(ITER_PAIRS):
                v_update(last=(it == ITER_PAIRS - 1))
                u_update(last=(it == ITER_PAIRS - 1))

            # ---------------- final assembly (host finishes) ----------------
            # out[:, 0] = sum_t (t2 - lnu)  (per partition)
            # out[:, 1] = sum_s lnv         (per partition)
            # out[:, 2] = max_s             (per partition)
            tt = sb.tile([128, NT], F32)
            nc.vector.tensor_sub(tt[:], t2[:], lnu[:])
            sums = sb.tile([128, 4], F32)
            nc.vector.memset(sums[:], 0.0)
            nc.vector.tensor_reduce(sums[:, 0:1], tt[:],
                                    axis=mybir.AxisListType.X, op=ALU.add)
            nc.vector.tensor_reduce(sums[:, 1:2], lnv[:],
                                    axis=mybir.AxisListType.X, op=ALU.add)
            nc.vector.tensor_reduce(sums[:, 2:3], mx[:],
                                    axis=mybir.AxisListType.X, op=ALU.max)
            nc.sync.dma_start(out_dram[:], sums[:])
    nc.compile()
    return nc


def _get_nc():
    if "nc" not in _CACHED:
        _CACHED["nc"] = _build()
    return _CACHED["nc"]


def kernel(features: np.ndarray, labels_vector: np.ndarray) -> np.ndarray:
    from concourse.bass_utils import run_bass_kernel_spmd

    F = np.ascontiguousarray(features, dtype=np.float32)
    L = np.ascontiguousarray(labels_vector, dtype=np.float32)
    assert F.shape == (N, D) and L.shape == (N, D)

    ltT = np.ascontiguousarray(L.T)                      # [300, 4096]
    ident = np.eye(128, dtype=np.float32)
    p = np.arange(128, dtype=np.float32)[:, None]

    in_maps = []
    for c in range(NCORES):
        rows = slice(c * R, (c + 1) * R)
        sig0 = (c * R + BATCH) % N
        rga = p + (c * R + 128 * np.arange(NT, dtype=np.float32)[None, :])
        in_maps.append({
            "ftT": np.ascontiguousarray(F[rows].T),      # [300, 512]
            "ltT": ltT,
            "fblk": F[rows],
            "lsig": np.ascontiguousarray(L[sig0:sig0 + R]),
            "rga": np.ascontiguousarray(rga),
            "ident": ident,
        })

    nc = _get_nc()
    res = run_bass_kernel_spmd(nc, in_maps, list(range(NCORES)))

    # host epilogue: finish the tiny partition reductions in fp32
    total = np.float32(0.0)
    max_s = np.float32(-np.inf)
    lnv_total = None
    for c in range(NCORES):
        o = res.results[c]["out"]                        # [128, 4] f32
        total = np.float32(total + np.float32(o[:, 0].sum(dtype=np.float32)))
        max_s = np.float32(max(max_s, np.float32(o[:, 2].max())))
        if c == 0:
            lnv_total = np.float32(o[:, 1].sum(dtype=np.float32))
    # loss = -(sum(logK_diag + log u) + sum(log v)) / N ; log v = -lnv
    total = np.float32(total - lnv_total)
    loss = np.float32(-total / np.float32(N))

    # fp32 overflow emulation: K = exp((s-1)*10) overflows for max_s > 9.872;
    # the fp32 reference then produces nan (inf in p0 -> normalize -> nan).
    with np.errstate(over="ignore"):
        k0max = np.exp(np.float32((max_s - np.float32(1.0)) * np.float32(10.0)),
                       dtype=np.float32)
    loss = np.float32(loss + np.float32(0.0) * k0max)
    return loss
